# revision 21
# baseline (speedup 1.0000x reference)
"""Trainium2 Bass kernel for nn_AttentionBlock (B=4, C=512, H=W=64).

Sharding: 8 cores = (batch b in 0..4) x (half of the N=4096 query pixels).
Each core computes a [2048, 4096] slice of its batch's attention map fully
on-chip (flash-style, never materialized to HBM), plus the Wq/Wk/Wv
projections for its batch.

Key layout trick: the attention is computed transposed, E[n, m] =
exp(logits[m, n]), directly from the matmul orientation (lhsT=k, rhs=qT),
and v is computed transposed too (vT[n, c], lhsT=xf, rhs=WvT), so the
attended = V @ attn^T matmul needs no on-chip transposes at all.

Precision strategy (PE moving-operand port is 2B/cycle/lane, so bf16
streams 2x faster than fp32/fp32r): the error-dominant q/k path runs in
fp32r (TF32-like), the bulk AV path (v, exp(attn)) in bf16, all
accumulation / softmax normalization / residual in fp32. The softmax
denominator sums exactly the bf16-rounded E the AV matmul consumes, so
normalization error cancels.

Per-core inputs are column-rotated copies of x[b] so the SPMD-shared
program always reads its own query columns at offset 0; the implied
permutation of the key/value axis n is applied consistently to k, vT and
the style mask, so results are unchanged (n is purely contracted).
"""

import os
import numpy as np

_B, _C, _H, _W = 4, 512, 64, 64
_N = _H * _W            # 4096 pixels
_C8 = _C // 8           # 64 qk channels
_M = _N // 2            # 2048 query rows per core
_P = 128

# module-level state for the optional trace mode (used by test.py)
LAST_EXEC_TIME_NS = None
_NC_CACHE = {}


def _split_waits(nc, mybir):
    """The walrus build in this container supports only ONE sync wait per
    instruction; move extras onto InstNoOp's inserted just before, on the
    same engine (semantically identical: waits execute serially)."""
    for f in nc.m.functions:
        for bb in f.blocks:
            new_insts = []
            for inst in bb.instructions:
                si = inst.sync_info
                if si is not None and si.on_wait and len(si.on_wait) > 1:
                    extras = si.on_wait[:-1]
                    si.on_wait = si.on_wait[-1:]
                    for w in extras:
                        nop = mybir.InstNoOp(
                            name=nc.get_next_instruction_name(),
                            ins=[], outs=[],
                            sync_info=mybir.SyncInfo(on_wait=[w], on_update=[]),
                            engine=inst.engine,
                        )
                        nc.register_instruction(nop)
                        new_insts.append(nop)
                new_insts.append(inst)
            bb.instructions[:] = new_insts


def _build_nc():
    from contextlib import ExitStack
    import concourse.bass as bass
    import concourse.tile as tile
    from concourse import mybir
    from concourse.bass import ts

    f32 = mybir.dt.float32
    f32r = mybir.dt.float32r
    bf16 = mybir.dt.bfloat16
    Exp = mybir.ActivationFunctionType.Exp

    nc = bass.Bass("TRN2", target_bir_lowering=False, debug=False)

    xf_r = nc.dram_tensor("xf_r", [_C, _N], f32r, kind="ExternalInput")
    wq = nc.dram_tensor("wq_t", [_C, _C8], f32r, kind="ExternalInput")
    wk = nc.dram_tensor("wk_t", [_C, _C8], f32r, kind="ExternalInput")
    wv = nc.dram_tensor("wv_t", [_C, _C], bf16, kind="ExternalInput")
    bq = nc.dram_tensor("bq", [_C8, 1], f32, kind="ExternalInput")
    bk = nc.dram_tensor("bk", [_C8, 1], f32, kind="ExternalInput")
    bv = nc.dram_tensor("bv128", [_P, _C], f32, kind="ExternalInput")
    mask = nc.dram_tensor("mask", [_P, _N // _P], bf16, kind="ExternalInput")
    # [128, 128] constant filled with 1/(gamma*num_styles)
    invgs = nc.dram_tensor("invgs", [_P, _P], f32, kind="ExternalInput")
    out = nc.dram_tensor("out_cm", [_C, _M], f32, kind="ExternalOutput")

    NCH = _N // _P      # 32 key chunks
    CJ = _C // _P       # 4 channel chunks
    MQ = _M // 512      # 4 query quarters per core

    with tile.TileContext(nc) as tc, ExitStack() as ctx:
        consts = ctx.enter_context(tc.tile_pool(name="consts", bufs=1))
        xfp = ctx.enter_context(tc.tile_pool(name="xfp", bufs=1))
        kvp = ctx.enter_context(tc.tile_pool(name="kvp", bufs=1))
        work = ctx.enter_context(tc.tile_pool(name="work", bufs=4))
        accp = ctx.enter_context(tc.tile_pool(name="accp", bufs=2))
        outp = ctx.enter_context(tc.tile_pool(name="outp", bufs=3))
        smallp = ctx.enter_context(tc.tile_pool(name="smallp", bufs=2))

        # ---- load phase ------------------------------------------------
        # small constants first so projections can start the moment the
        # bulk x chunks land
        wq_sb = consts.tile([_P, CJ, _C8], f32r)
        wk_sb = consts.tile([_P, CJ, _C8], f32r)
        wv_sb = consts.tile([_P, CJ, _C], bf16)
        for j in range(CJ):
            nc.sync.dma_start(out=wq_sb[:, j, :], in_=wq[j * _P:(j + 1) * _P, :])
            nc.sync.dma_start(out=wk_sb[:, j, :], in_=wk[j * _P:(j + 1) * _P, :])
            nc.sync.dma_start(out=wv_sb[:, j, :], in_=wv[j * _P:(j + 1) * _P, :])
        bq_sb = consts.tile([_C8, 1], f32)
        bk_sb = consts.tile([_C8, 1], f32)
        bv_sb = consts.tile([_P, _C], f32)
        mask_sb = consts.tile([_P, NCH], bf16)
        invgs_sb = consts.tile([_P, _P], f32)
        nc.sync.dma_start(out=bq_sb, in_=bq[:, :])
        nc.sync.dma_start(out=bk_sb, in_=bk[:, :])
        nc.sync.dma_start(out=bv_sb, in_=bv[:, :])
        nc.sync.dma_start(out=mask_sb, in_=mask[:, :])
        nc.sync.dma_start(out=invgs_sb, in_=invgs[:, :])

        # x arrives in 8 column-blocks so projections, the bf16 cast and the
        # first mq pass can all pipeline behind the DMA stream
        xr_sb = xfp.tile([_P, CJ, _N], f32r)
        xb_sb = xfp.tile([_P, CJ, _N], bf16)
        NB = _N // 512
        for nb in range(NB):
            for j in range(CJ):
                nc.sync.dma_start(
                    out=xr_sb[:, j, ts(nb, 512)],
                    in_=xf_r[j * _P:(j + 1) * _P, ts(nb, 512)],
                )
            # derive the bf16 copy on-device (DVE converts on copy),
            # saving 4 MiB of HBM input traffic
            nc.vector.tensor_copy(
                out=xb_sb[:, :, ts(nb, 512)],
                in_=xr_sb[:, :, ts(nb, 512)].bitcast(f32),
            )

        # k is stored row-PACKED for the QK matmuls: even n-chunks on
        # partitions 0..63, odd n-chunks on partitions 64..127, so two K=64
        # QK matmuls run CONCURRENTLY in separate PE row-groups (~1.9x).
        # qT is duplicated onto both partition halves to feed them.
        k2_sb = kvp.tile([_P, NCH // 2, _P], f32r)
        ktmp = kvp.tile([_C8, NCH // 2, _P], f32r)
        qt_sb = kvp.tile([_P, _M], f32r)
        vt_sb = kvp.tile([_P, NCH, _C], bf16)

        # ---- projections ----------------------------------------------
        with tc.tile_pool(name="pp_proj", bufs=2, space="PSUM") as pp_proj:
            # k[o, n] = Wk @ xf + bk  (o on partitions)
            for t8 in range(_N // 512):
                ps = pp_proj.tile([_C8, 512], f32, tag="pk")
                for j in range(CJ):
                    nc.tensor.matmul(
                        ps,
                        wk_sb[:, j, :],
                        xr_sb[:, j, ts(t8, 512)],
                        start=(j == 0), stop=(j == CJ - 1),
                    )
                # scatter the 4 n-chunks of this psum tile into the packed
                # layout: even chunks straight to the low half, odd chunks
                # via a staging tile + partition-shifting SBUF DMA (compute
                # engines are lane-aligned and cannot shift partitions)
                for i in range(4):
                    c = 4 * t8 + i
                    if c % 2 == 0:
                        nc.scalar.add(k2_sb[0:_C8, c // 2, :],
                                      ps[:, ts(i, _P)], bk_sb)
                    else:
                        nc.scalar.add(ktmp[:, c // 2, :],
                                      ps[:, ts(i, _P)], bk_sb)
                        nc.sync.dma_start(
                            out=k2_sb[_C8:2 * _C8, c // 2, :],
                            in_=ktmp[:, c // 2, :],
                        )
            # qT[o, m] = Wq @ xf[:, :2048] + bq, duplicated onto both halves
            for t4 in range(_M // 512):
                ps = pp_proj.tile([_C8, 512], f32, tag="pk")
                for j in range(CJ):
                    nc.tensor.matmul(
                        ps,
                        wq_sb[:, j, :],
                        xr_sb[:, j, ts(t4, 512)],
                        start=(j == 0), stop=(j == CJ - 1),
                    )
                nc.scalar.add(qt_sb[0:_C8, ts(t4, 512)], ps, bq_sb)
                nc.sync.dma_start(
                    out=qt_sb[_C8:2 * _C8, ts(t4, 512)],
                    in_=qt_sb[0:_C8, ts(t4, 512)],
                )

        # ---- attention main loop --------------------------------------
        with tc.tile_pool(name="pp_e", bufs=2, space="PSUM") as pp_e, \
             tc.tile_pool(name="pp_att", bufs=4, space="PSUM") as pp_att:
            for mq in range(MQ):
                atts = [
                    pp_att.tile([_P, 512], f32, tag="att", name=f"att_{mq}_{cc}")
                    for cc in range(CJ)
                ]
                # rowsum accumulators, split across DVE (even chunks) and the
                # otherwise-idle GpSimd (odd chunks) so neither chain paces
                # the loop
                acc_d = accp.tile([_P, 512], f32, tag="acc_d")
                acc_g = accp.tile([_P, 512], f32, tag="acc_g")
                nc.vector.memset(acc_d, 0.0)
                nc.gpsimd.memset(acc_g, 0.0)
                for tp in range(NCH // 2):
                    if mq == 0:
                        # vT[n, c] = xf^T @ Wv^T + bv for both chunks of the
                        # pair, produced just-in-time for its AV matmuls
                        # (hides the whole vT projection in the first mq pass)
                        vps2 = pp_e.tile([_P, 2, _C], f32, tag="eps",
                                         name=f"vps_{tp}")
                        for half in (0, 1):
                            for j in range(CJ):
                                nc.tensor.matmul(
                                    vps2[:, half, :],
                                    xb_sb[:, j, ts(2 * tp + half, _P)],
                                    wv_sb[:, j, :],
                                    start=(j == 0), stop=(j == CJ - 1),
                                )
                            nc.vector.tensor_add(
                                vt_sb[:, 2 * tp + half, :],
                                vps2[:, half, :], bv_sb)
                    # two K=64 QK matmuls run concurrently in PE row-groups
                    # 0-1 (k2 low half) and 2-3 (k2 high half), into the two
                    # banks of one paired PSUM tile -> ONE wide exp
                    e2_ps = pp_e.tile([_P, 2, 512], f32, tag="eps",
                                      name=f"eps_{mq}_{tp}")
                    for half in (0, 1):
                        nc.tensor.matmul(
                            e2_ps[:, half, :],
                            k2_sb[half * _C8:(half + 1) * _C8, tp, :],
                            qt_sb[half * _C8:(half + 1) * _C8, ts(mq, 512)],
                            start=True, stop=True,
                        )
                    e2_sb = work.tile([_P, 2, 512], bf16, tag="esb", bufs=3)
                    nc.scalar.activation(e2_sb, e2_ps, Exp)
                    # rowsum accumulation must see E BEFORE masking (softmax
                    # denominator includes the masked element)
                    nc.vector.tensor_add(acc_d, acc_d, e2_sb[:, 0, :])
                    nc.gpsimd.tensor_add(acc_g, acc_g, e2_sb[:, 1, :])
                    if mq == 0:
                        # style mask: zero E[s_b, m=0] (data-driven; all-ones
                        # for cores not owning query row 0)
                        for half in (0, 1):
                            t = 2 * tp + half
                            nc.vector.tensor_mul(
                                e2_sb[:, half, 0:1], e2_sb[:, half, 0:1],
                                mask_sb[:, t:t + 1]
                            )
                    for half in (0, 1):
                        t = 2 * tp + half
                        for cc in range(CJ):
                            nc.tensor.matmul(
                                atts[cc],
                                vt_sb[:, t, ts(cc, _P)],
                                e2_sb[:, half, :],
                                start=(t == 0), stop=(t == NCH - 1),
                            )
                # Stage attended to SBUF on ScalarE immediately, freeing the
                # four PSUM banks so the next mq's AV matmuls aren't blocked
                # behind the (slow) normalization chain.
                att_cp = []
                for cc in range(CJ):
                    ac = outp.tile([_P, 512], f32, tag="att_cp",
                                   name=f"attcp_{mq}_{cc}", bufs=4)
                    nc.scalar.copy(ac, atts[cc])
                    att_cp.append(ac)
                # Fused partition-reduce + broadcast + un-scale:
                # rb[p, m] = sum_n invgs * (acc_d + acc_g)[n, m]
                #          = rowsum[m]/(gamma*NS)   (full fp32 matmuls)
                rb_ps = pp_e.tile([_P, 512], f32, tag="eps", name=f"rb_{mq}")
                nc.tensor.matmul(rb_ps, invgs_sb, acc_d, start=True, stop=False)
                nc.tensor.matmul(rb_ps, invgs_sb, acc_g, start=False, stop=True)
                recipb = smallp.tile([_P, 512], f32, tag="recipb")
                nc.vector.reciprocal(recipb, rb_ps)
                for cc in range(CJ):
                    o = outp.tile([_P, 512], f32, tag="o")
                    nc.vector.tensor_mul(o, att_cp[cc], recipb)
                    nc.vector.tensor_add(o, o, xr_sb[:, cc, ts(mq, 512)].bitcast(f32))
                    nc.sync.dma_start(
                        out=out[ts(cc, _P), ts(mq, 512)], in_=o
                    )

    _split_waits(nc, mybir)
    return nc


def _get_nc():
    if "nc" not in _NC_CACHE:
        _NC_CACHE["nc"] = _build_nc()
    return _NC_CACHE["nc"]


def _install_trace_shim():
    """Optional: enable NTFF profiling under axon (test.py sets
    KERNEL_TRACE=1). Registers the ctypes NTFF hook that the container's
    antenv lacks, and stubs out the artifact upload."""
    import sys, types
    if "antenv.axon_hooks" not in sys.modules:
        mod = types.ModuleType("antenv.axon_hooks")
        _h = [None]
        mod.set_axon_ntff_profile_hook = lambda h: _h.__setitem__(0, h)
        mod.get_axon_ntff_profile_hook = lambda: _h[0]
        sys.modules["antenv.axon_hooks"] = mod
    import antenv.axon_hooks as ah
    if ah.get_axon_ntff_profile_hook() is None:
        try:
            from trn_agent_boot.trn_boot import _ntff_profile_via_ctypes
            ah.set_axon_ntff_profile_hook(
                _ntff_profile_via_ctypes("/opt/axon/libaxon_pjrt.so")
            )
        except Exception:
            pass
    import concourse.bass_utils as bu
    bu.upload_artifacts = lambda tmpdir: "local://" + tmpdir


def kernel(x, Wq, bq, Wk, bk, Wv, bv, gamma, style_idx, num_styles):
    global LAST_EXEC_TIME_NS
    import ml_dtypes
    from concourse.bass_utils import run_bass_kernel_spmd

    x = np.asarray(x, dtype=np.float32)
    Wq = np.asarray(Wq, dtype=np.float32)
    Wk = np.asarray(Wk, dtype=np.float32)
    Wv = np.asarray(Wv, dtype=np.float32)
    bq = np.asarray(bq, dtype=np.float32).reshape(_C8, 1)
    bk = np.asarray(bk, dtype=np.float32).reshape(_C8, 1)
    bv = np.asarray(bv, dtype=np.float32).reshape(1, _C)
    gamma = np.asarray(gamma, dtype=np.float32).reshape(-1)
    style_idx = np.asarray(style_idx).reshape(-1).astype(np.int64)
    NS = int(np.asarray(num_styles))
    assert NS == _N, f"kernel specialized for num_styles == {_N}, got {NS}"
    B, C, H, W = x.shape
    assert (B, C, H, W) == (_B, _C, _H, _W)

    gscale = float(gamma[0]) * float(NS)
    inv = 1.0 / gscale if gscale != 0.0 else 1e30
    invgs = np.full((_P, _P), inv, dtype=np.float32)
    wqT = np.ascontiguousarray(Wq.T)                          # [C, C8] f32
    wkT = np.ascontiguousarray(Wk.T)
    wvT = np.ascontiguousarray(Wv.T.astype(ml_dtypes.bfloat16))
    bv128 = np.ascontiguousarray(np.broadcast_to(bv, (_P, _C)))

    xf_full = x.reshape(B, C, _N)
    in_maps = []
    for core in range(8):
        b, half = divmod(core, 2)
        xr = xf_full[b]
        if half:
            xr = np.concatenate([xr[:, _M:], xr[:, :_M]], axis=1)
        xr = np.ascontiguousarray(xr)
        m = np.ones((_P, _N // _P), dtype=np.float32)
        if half == 0:
            s = int(style_idx[b])
            m[s % _P, s // _P] = 0.0
        in_maps.append({
            "xf_r": xr,
            "wq_t": wqT, "wk_t": wkT, "wv_t": wvT,
            "bq": bq, "bk": bk, "bv128": bv128,
            "mask": m.astype(ml_dtypes.bfloat16), "invgs": invgs,
        })

    trace = os.environ.get("KERNEL_TRACE", "0") == "1"
    if trace:
        _install_trace_shim()

    nc = _get_nc()
    kw = {}
    tdir = os.environ.get("KERNEL_TRACE_DIR")
    if trace and tdir:
        os.makedirs(tdir, exist_ok=True)
        kw["tmpdir"] = tdir
    res = run_bass_kernel_spmd(
        nc, in_maps, core_ids=list(range(8)), trace=trace, **kw
    )
    LAST_EXEC_TIME_NS = res.exec_time_ns

    out = np.empty((B, C, _N), dtype=np.float32)
    for core in range(8):
        b, half = divmod(core, 2)
        out[b, :, half * _M:(half + 1) * _M] = res.results[core]["out_cm"]
    return out.reshape(B, C, H, W)


# revision 22
# speedup vs baseline: 1.0966x; 1.0966x over previous
"""Trainium2 Bass kernel for nn_AttentionBlock (B=4, C=512, H=W=64).

Sharding: 8 cores = (batch b in 0..4) x (half of the N=4096 query pixels).
Each core computes a [2048, 4096] slice of its batch's attention map fully
on-chip (flash-style, never materialized to HBM), plus the Wq/Wk/Wv
projections for its batch.

Key layout trick: the attention is computed transposed, E[n, m] =
exp(logits[m, n]), directly from the matmul orientation (lhsT=k, rhs=qT),
and v is computed transposed too (vT[n, c], lhsT=xf, rhs=WvT), so the
attended = V @ attn^T matmul needs no on-chip transposes at all.

Precision strategy (PE moving-operand port is 2B/cycle/lane, so bf16
streams 2x faster than fp32/fp32r): the error-dominant q/k path runs in
fp32r (TF32-like), the bulk AV path (v, exp(attn)) in bf16, all
accumulation / softmax normalization / residual in fp32. The softmax
denominator sums exactly the bf16-rounded E the AV matmul consumes, so
normalization error cancels.

Per-core inputs are column-rotated copies of x[b] so the SPMD-shared
program always reads its own query columns at offset 0; the implied
permutation of the key/value axis n is applied consistently to k, vT and
the style mask, so results are unchanged (n is purely contracted).
"""

import os
import numpy as np

_B, _C, _H, _W = 4, 512, 64, 64
_N = _H * _W            # 4096 pixels
_C8 = _C // 8           # 64 qk channels
_M = _N // 2            # 2048 query rows per core
_P = 128

# module-level state for the optional trace mode (used by test.py)
LAST_EXEC_TIME_NS = None
_NC_CACHE = {}


def _split_waits(nc, mybir):
    """The walrus build in this container supports only ONE sync wait per
    instruction; move extras onto InstNoOp's inserted just before, on the
    same engine (semantically identical: waits execute serially)."""
    for f in nc.m.functions:
        for bb in f.blocks:
            new_insts = []
            for inst in bb.instructions:
                si = inst.sync_info
                if si is not None and si.on_wait and len(si.on_wait) > 1:
                    extras = si.on_wait[:-1]
                    si.on_wait = si.on_wait[-1:]
                    for w in extras:
                        nop = mybir.InstNoOp(
                            name=nc.get_next_instruction_name(),
                            ins=[], outs=[],
                            sync_info=mybir.SyncInfo(on_wait=[w], on_update=[]),
                            engine=inst.engine,
                        )
                        nc.register_instruction(nop)
                        new_insts.append(nop)
                new_insts.append(inst)
            bb.instructions[:] = new_insts


def _build_nc():
    from contextlib import ExitStack
    import concourse.bass as bass
    import concourse.tile as tile
    from concourse import mybir
    from concourse.bass import ts

    f32 = mybir.dt.float32
    f32r = mybir.dt.float32r
    bf16 = mybir.dt.bfloat16
    Exp = mybir.ActivationFunctionType.Exp

    nc = bass.Bass("TRN2", target_bir_lowering=False, debug=False)

    xf_r = nc.dram_tensor("xf_r", [_C, _N], f32r, kind="ExternalInput")
    wq = nc.dram_tensor("wq_t", [_C, _C8], f32r, kind="ExternalInput")
    wk = nc.dram_tensor("wk_t", [_C, _C8], f32r, kind="ExternalInput")
    wv = nc.dram_tensor("wv_t", [_C, _C], bf16, kind="ExternalInput")
    bq = nc.dram_tensor("bq", [_C8, 1], f32, kind="ExternalInput")
    bk = nc.dram_tensor("bk", [_C8, 1], f32, kind="ExternalInput")
    bv = nc.dram_tensor("bv128", [_P, _C], f32, kind="ExternalInput")
    mask = nc.dram_tensor("mask", [_P, _N // _P], bf16, kind="ExternalInput")
    # [128, 128] constant filled with 1/(gamma*num_styles)
    invgs = nc.dram_tensor("invgs", [_P, _P], f32, kind="ExternalInput")
    out = nc.dram_tensor("out_cm", [_C, _M], f32, kind="ExternalOutput")

    NCH = _N // _P      # 32 key chunks
    CJ = _C // _P       # 4 channel chunks
    MQ = _M // 512      # 4 query quarters per core

    with tile.TileContext(nc) as tc, ExitStack() as ctx:
        consts = ctx.enter_context(tc.tile_pool(name="consts", bufs=1))
        xfp = ctx.enter_context(tc.tile_pool(name="xfp", bufs=1))
        kvp = ctx.enter_context(tc.tile_pool(name="kvp", bufs=1))
        work = ctx.enter_context(tc.tile_pool(name="work", bufs=4))
        accp = ctx.enter_context(tc.tile_pool(name="accp", bufs=2))
        outp = ctx.enter_context(tc.tile_pool(name="outp", bufs=3))
        smallp = ctx.enter_context(tc.tile_pool(name="smallp", bufs=2))

        # ---- load phase ------------------------------------------------
        # small constants first so projections can start the moment the
        # bulk x chunks land
        wq_sb = consts.tile([_P, CJ, _C8], f32r)
        wk_sb = consts.tile([_P, CJ, _C8], f32r)
        wv_sb = consts.tile([_P, CJ, _C], bf16)
        for j in range(CJ):
            nc.sync.dma_start(out=wq_sb[:, j, :], in_=wq[j * _P:(j + 1) * _P, :])
            nc.sync.dma_start(out=wk_sb[:, j, :], in_=wk[j * _P:(j + 1) * _P, :])
            nc.sync.dma_start(out=wv_sb[:, j, :], in_=wv[j * _P:(j + 1) * _P, :])
        bq_sb = consts.tile([_C8, 1], f32)
        bk_sb = consts.tile([_C8, 1], f32)
        bv_sb = consts.tile([_P, _C], f32)
        mask_sb = consts.tile([_P, NCH], bf16)
        invgs_sb = consts.tile([_P, _P], f32)
        nc.sync.dma_start(out=bq_sb, in_=bq[:, :])
        nc.sync.dma_start(out=bk_sb, in_=bk[:, :])
        nc.sync.dma_start(out=bv_sb, in_=bv[:, :])
        nc.sync.dma_start(out=mask_sb, in_=mask[:, :])
        nc.sync.dma_start(out=invgs_sb, in_=invgs[:, :])

        # x arrives in 8 column-blocks so projections, the bf16 cast and the
        # first mq pass can all pipeline behind the DMA stream
        xr_sb = xfp.tile([_P, CJ, _N], f32r)
        xb_sb = xfp.tile([_P, CJ, _N], bf16)
        NB = _N // 512
        for nb in range(NB):
            for j in range(CJ):
                nc.sync.dma_start(
                    out=xr_sb[:, j, ts(nb, 512)],
                    in_=xf_r[j * _P:(j + 1) * _P, ts(nb, 512)],
                )
            # derive the bf16 copy on-device (DVE converts on copy),
            # saving 4 MiB of HBM input traffic
            nc.vector.tensor_copy(
                out=xb_sb[:, :, ts(nb, 512)],
                in_=xr_sb[:, :, ts(nb, 512)].bitcast(f32),
            )

        # k is stored row-PACKED for the QK matmuls: even n-chunks on
        # partitions 0..63, odd n-chunks on partitions 64..127, so two K=64
        # QK matmuls run CONCURRENTLY in separate PE row-groups (~1.9x).
        # qT is duplicated onto both partition halves to feed them.
        k2_sb = kvp.tile([_P, NCH // 2, _P], f32r)
        ktmp = kvp.tile([_C8, NCH // 2, _P], f32r)
        qt_sb = kvp.tile([_P, _M], f32r)
        vt_sb = kvp.tile([_P, NCH, _C], bf16)

        # ---- projections ----------------------------------------------
        with tc.tile_pool(name="pp_proj", bufs=2, space="PSUM") as pp_proj:
            # k[o, n] = Wk @ xf + bk  (o on partitions)
            for t8 in range(_N // 512):
                ps = pp_proj.tile([_C8, 512], f32, tag="pk")
                for j in range(CJ):
                    nc.tensor.matmul(
                        ps,
                        wk_sb[:, j, :],
                        xr_sb[:, j, ts(t8, 512)],
                        start=(j == 0), stop=(j == CJ - 1),
                    )
                # scatter the 4 n-chunks of this psum tile into the packed
                # layout: even chunks straight to the low half, odd chunks
                # via a staging tile + partition-shifting SBUF DMA (compute
                # engines are lane-aligned and cannot shift partitions)
                for i in range(4):
                    c = 4 * t8 + i
                    if c % 2 == 0:
                        nc.scalar.add(k2_sb[0:_C8, c // 2, :],
                                      ps[:, ts(i, _P)], bk_sb)
                    else:
                        nc.scalar.add(ktmp[:, c // 2, :],
                                      ps[:, ts(i, _P)], bk_sb)
                        nc.sync.dma_start(
                            out=k2_sb[_C8:2 * _C8, c // 2, :],
                            in_=ktmp[:, c // 2, :],
                        )
            # qT[o, m] = Wq @ xf[:, :2048] + bq, duplicated onto both halves
            for t4 in range(_M // 512):
                ps = pp_proj.tile([_C8, 512], f32, tag="pk")
                for j in range(CJ):
                    nc.tensor.matmul(
                        ps,
                        wq_sb[:, j, :],
                        xr_sb[:, j, ts(t4, 512)],
                        start=(j == 0), stop=(j == CJ - 1),
                    )
                nc.scalar.add(qt_sb[0:_C8, ts(t4, 512)], ps, bq_sb)
                nc.sync.dma_start(
                    out=qt_sb[_C8:2 * _C8, ts(t4, 512)],
                    in_=qt_sb[0:_C8, ts(t4, 512)],
                )

        # ---- attention main loop --------------------------------------
        with tc.tile_pool(name="pp_e", bufs=4, space="PSUM") as pp_e, \
             tc.tile_pool(name="pp_att", bufs=4, space="PSUM") as pp_att:
            for mq in range(MQ):
                atts = [
                    pp_att.tile([_P, 512], f32, tag="att", name=f"att_{mq}_{cc}")
                    for cc in range(CJ)
                ]
                # rowsum accumulators, split across DVE (even chunks) and the
                # otherwise-idle GpSimd (odd chunks) so neither chain paces
                # the loop
                acc_d = accp.tile([_P, 512], f32, tag="acc_d")
                acc_g = accp.tile([_P, 512], f32, tag="acc_g")
                nc.vector.memset(acc_d, 0.0)
                nc.gpsimd.memset(acc_g, 0.0)
                for tp in range(NCH // 2):
                    if mq == 0:
                        # vT[n, c] = xf^T @ Wv^T + bv, produced just-in-time
                        # for this pair's AV matmuls (hides the whole vT
                        # projection inside the first mq pass)
                        for t in (2 * tp, 2 * tp + 1):
                            vps = pp_e.tile([_P, _C], f32, tag="eps",
                                            name=f"vps_{t}")
                            for j in range(CJ):
                                nc.tensor.matmul(
                                    vps,
                                    xb_sb[:, j, ts(t, _P)],
                                    wv_sb[:, j, :],
                                    start=(j == 0), stop=(j == CJ - 1),
                                )
                            nc.vector.tensor_add(vt_sb[:, t, :], vps, bv_sb)
                    # two K=64 QK matmuls run concurrently in PE row-groups
                    # 0-1 (k2 low half) and 2-3 (k2 high half)
                    e_pair = []
                    for half in (0, 1):
                        e_ps = pp_e.tile([_P, 512], f32, tag="eps",
                                         name=f"eps_{mq}_{tp}_{half}")
                        nc.tensor.matmul(
                            e_ps,
                            k2_sb[half * _C8:(half + 1) * _C8, tp, :],
                            qt_sb[half * _C8:(half + 1) * _C8, ts(mq, 512)],
                            start=True, stop=True,
                        )
                        e_pair.append(e_ps)
                    for half in (0, 1):
                        t = 2 * tp + half
                        e_sb = work.tile([_P, 512], bf16, tag="esb", bufs=6)
                        nc.scalar.activation(e_sb, e_pair[half], Exp)
                        # rowsum accumulation must see E BEFORE masking
                        # (softmax denominator includes the masked element)
                        if half == 0:
                            nc.vector.tensor_add(acc_d, acc_d, e_sb)
                        else:
                            nc.gpsimd.tensor_add(acc_g, acc_g, e_sb)
                        if mq == 0:
                            # style mask: zero E[s_b, m=0] (data-driven;
                            # all-ones for cores not owning query row 0)
                            nc.vector.tensor_mul(
                                e_sb[:, 0:1], e_sb[:, 0:1], mask_sb[:, t:t + 1]
                            )
                        for cc in range(CJ):
                            nc.tensor.matmul(
                                atts[cc],
                                vt_sb[:, t, ts(cc, _P)],
                                e_sb,
                                start=(t == 0), stop=(t == NCH - 1),
                            )
                # Stage attended to SBUF on ScalarE immediately, freeing the
                # four PSUM banks so the next mq's AV matmuls aren't blocked
                # behind the (slow) normalization chain.
                att_cp = []
                for cc in range(CJ):
                    ac = outp.tile([_P, 512], f32, tag="att_cp",
                                   name=f"attcp_{mq}_{cc}", bufs=4)
                    nc.scalar.copy(ac, atts[cc])
                    att_cp.append(ac)
                # Fused partition-reduce + broadcast + un-scale:
                # rb[p, m] = sum_n invgs * (acc_d + acc_g)[n, m]
                #          = rowsum[m]/(gamma*NS)   (full fp32 matmuls)
                rb_ps = pp_e.tile([_P, 512], f32, tag="eps", name=f"rb_{mq}")
                nc.tensor.matmul(rb_ps, invgs_sb, acc_d, start=True, stop=False)
                nc.tensor.matmul(rb_ps, invgs_sb, acc_g, start=False, stop=True)
                recipb = smallp.tile([_P, 512], f32, tag="recipb")
                nc.vector.reciprocal(recipb, rb_ps)
                for cc in range(CJ):
                    o = outp.tile([_P, 512], f32, tag="o")
                    nc.vector.tensor_mul(o, att_cp[cc], recipb)
                    nc.vector.tensor_add(o, o, xr_sb[:, cc, ts(mq, 512)].bitcast(f32))
                    nc.sync.dma_start(
                        out=out[ts(cc, _P), ts(mq, 512)], in_=o
                    )

    _split_waits(nc, mybir)
    return nc


def _get_nc():
    if "nc" not in _NC_CACHE:
        _NC_CACHE["nc"] = _build_nc()
    return _NC_CACHE["nc"]


def _install_trace_shim():
    """Optional: enable NTFF profiling under axon (test.py sets
    KERNEL_TRACE=1). Registers the ctypes NTFF hook that the container's
    antenv lacks, and stubs out the artifact upload."""
    import sys, types
    if "antenv.axon_hooks" not in sys.modules:
        mod = types.ModuleType("antenv.axon_hooks")
        _h = [None]
        mod.set_axon_ntff_profile_hook = lambda h: _h.__setitem__(0, h)
        mod.get_axon_ntff_profile_hook = lambda: _h[0]
        sys.modules["antenv.axon_hooks"] = mod
    import antenv.axon_hooks as ah
    if ah.get_axon_ntff_profile_hook() is None:
        try:
            from trn_agent_boot.trn_boot import _ntff_profile_via_ctypes
            ah.set_axon_ntff_profile_hook(
                _ntff_profile_via_ctypes("/opt/axon/libaxon_pjrt.so")
            )
        except Exception:
            pass
    import concourse.bass_utils as bu
    bu.upload_artifacts = lambda tmpdir: "local://" + tmpdir


def kernel(x, Wq, bq, Wk, bk, Wv, bv, gamma, style_idx, num_styles):
    global LAST_EXEC_TIME_NS
    import ml_dtypes
    from concourse.bass_utils import run_bass_kernel_spmd

    x = np.asarray(x, dtype=np.float32)
    Wq = np.asarray(Wq, dtype=np.float32)
    Wk = np.asarray(Wk, dtype=np.float32)
    Wv = np.asarray(Wv, dtype=np.float32)
    bq = np.asarray(bq, dtype=np.float32).reshape(_C8, 1)
    bk = np.asarray(bk, dtype=np.float32).reshape(_C8, 1)
    bv = np.asarray(bv, dtype=np.float32).reshape(1, _C)
    gamma = np.asarray(gamma, dtype=np.float32).reshape(-1)
    style_idx = np.asarray(style_idx).reshape(-1).astype(np.int64)
    NS = int(np.asarray(num_styles))
    assert NS == _N, f"kernel specialized for num_styles == {_N}, got {NS}"
    B, C, H, W = x.shape
    assert (B, C, H, W) == (_B, _C, _H, _W)

    gscale = float(gamma[0]) * float(NS)
    inv = 1.0 / gscale if gscale != 0.0 else 1e30
    invgs = np.full((_P, _P), inv, dtype=np.float32)
    wqT = np.ascontiguousarray(Wq.T)                          # [C, C8] f32
    wkT = np.ascontiguousarray(Wk.T)
    wvT = np.ascontiguousarray(Wv.T.astype(ml_dtypes.bfloat16))
    bv128 = np.ascontiguousarray(np.broadcast_to(bv, (_P, _C)))

    xf_full = x.reshape(B, C, _N)
    in_maps = []
    for core in range(8):
        b, half = divmod(core, 2)
        xr = xf_full[b]
        if half:
            xr = np.concatenate([xr[:, _M:], xr[:, :_M]], axis=1)
        xr = np.ascontiguousarray(xr)
        m = np.ones((_P, _N // _P), dtype=np.float32)
        if half == 0:
            s = int(style_idx[b])
            m[s % _P, s // _P] = 0.0
        in_maps.append({
            "xf_r": xr,
            "wq_t": wqT, "wk_t": wkT, "wv_t": wvT,
            "bq": bq, "bk": bk, "bv128": bv128,
            "mask": m.astype(ml_dtypes.bfloat16), "invgs": invgs,
        })

    trace = os.environ.get("KERNEL_TRACE", "0") == "1"
    if trace:
        _install_trace_shim()

    nc = _get_nc()
    kw = {}
    tdir = os.environ.get("KERNEL_TRACE_DIR")
    if trace and tdir:
        os.makedirs(tdir, exist_ok=True)
        kw["tmpdir"] = tdir
    res = run_bass_kernel_spmd(
        nc, in_maps, core_ids=list(range(8)), trace=trace, **kw
    )
    LAST_EXEC_TIME_NS = res.exec_time_ns

    out = np.empty((B, C, _N), dtype=np.float32)
    for core in range(8):
        b, half = divmod(core, 2)
        out[b, :, half * _M:(half + 1) * _M] = res.results[core]["out_cm"]
    return out.reshape(B, C, H, W)


# revision 24
# speedup vs baseline: 1.2936x; 1.1797x over previous
"""Trainium2 Bass kernel for nn_AttentionBlock (B=4, C=512, H=W=64).

Sharding: 8 cores = (batch b in 0..4) x (half of the N=4096 query pixels).
Each core computes a [2048, 4096] slice of its batch's attention map fully
on-chip (flash-style, never materialized to HBM), plus the Wq/Wk/Wv
projections for its batch.

Key layout trick: the attention is computed transposed, E[n, m] =
exp(logits[m, n]), directly from the matmul orientation (lhsT=k, rhs=qT),
and v is computed transposed too (vT[n, c], lhsT=xf, rhs=WvT), so the
attended = V @ attn^T matmul needs no on-chip transposes at all.

Precision strategy (PE moving-operand port is 2B/cycle/lane, so bf16
streams 2x faster than fp32/fp32r): the error-dominant q/k path runs in
fp32r (TF32-like), the bulk AV path (v, exp(attn)) in bf16, all
accumulation / softmax normalization / residual in fp32. The softmax
denominator sums exactly the bf16-rounded E the AV matmul consumes, so
normalization error cancels.

Per-core inputs are column-rotated copies of x[b] so the SPMD-shared
program always reads its own query columns at offset 0; the implied
permutation of the key/value axis n is applied consistently to k, vT and
the style mask, so results are unchanged (n is purely contracted).
"""

import os
import numpy as np

_B, _C, _H, _W = 4, 512, 64, 64
_N = _H * _W            # 4096 pixels
_C8 = _C // 8           # 64 qk channels
_M = _N // 2            # 2048 query rows per core
_P = 128

# module-level state for the optional trace mode (used by test.py)
LAST_EXEC_TIME_NS = None
_NC_CACHE = {}


def _split_waits(nc, mybir):
    """The walrus build in this container supports only ONE sync wait per
    instruction; move extras onto InstNoOp's inserted just before, on the
    same engine (semantically identical: waits execute serially)."""
    for f in nc.m.functions:
        for bb in f.blocks:
            new_insts = []
            for inst in bb.instructions:
                si = inst.sync_info
                if si is not None and si.on_wait and len(si.on_wait) > 1:
                    extras = si.on_wait[:-1]
                    si.on_wait = si.on_wait[-1:]
                    for w in extras:
                        nop = mybir.InstNoOp(
                            name=nc.get_next_instruction_name(),
                            ins=[], outs=[],
                            sync_info=mybir.SyncInfo(on_wait=[w], on_update=[]),
                            engine=inst.engine,
                        )
                        nc.register_instruction(nop)
                        new_insts.append(nop)
                new_insts.append(inst)
            bb.instructions[:] = new_insts


def _build_nc():
    from contextlib import ExitStack
    import concourse.bass as bass
    import concourse.tile as tile
    from concourse import mybir
    from concourse.bass import ts

    f32 = mybir.dt.float32
    f32r = mybir.dt.float32r
    bf16 = mybir.dt.bfloat16
    Exp = mybir.ActivationFunctionType.Exp

    nc = bass.Bass("TRN2", target_bir_lowering=False, debug=False)

    xf_r = nc.dram_tensor("xf_r", [_C, _N], f32r, kind="ExternalInput")
    wq = nc.dram_tensor("wq_t", [_C, _C8], f32r, kind="ExternalInput")
    wk = nc.dram_tensor("wk_t", [_C, _C8], f32r, kind="ExternalInput")
    wv = nc.dram_tensor("wv_t", [_C, _C], bf16, kind="ExternalInput")
    bq = nc.dram_tensor("bq", [_P, 1], f32, kind="ExternalInput")
    bk = nc.dram_tensor("bk", [_P, 1], f32, kind="ExternalInput")
    bv = nc.dram_tensor("bv128", [_P, _C], f32, kind="ExternalInput")
    mask = nc.dram_tensor("mask", [_P, _N // _P], bf16, kind="ExternalInput")
    # [128, 128] constant filled with 1/(gamma*num_styles)
    invgs = nc.dram_tensor("invgs", [_P, _P], f32, kind="ExternalInput")
    out = nc.dram_tensor("out_cm", [_C, _M], f32, kind="ExternalOutput")

    NCH = _N // _P      # 32 key chunks
    CJ = _C // _P       # 4 channel chunks
    MQ = _M // 512      # 4 query quarters per core

    with tile.TileContext(nc) as tc, ExitStack() as ctx:
        consts = ctx.enter_context(tc.tile_pool(name="consts", bufs=1))
        xfp = ctx.enter_context(tc.tile_pool(name="xfp", bufs=1))
        kvp = ctx.enter_context(tc.tile_pool(name="kvp", bufs=1))
        work = ctx.enter_context(tc.tile_pool(name="work", bufs=4))
        accp = ctx.enter_context(tc.tile_pool(name="accp", bufs=2))
        outp = ctx.enter_context(tc.tile_pool(name="outp", bufs=3))
        smallp = ctx.enter_context(tc.tile_pool(name="smallp", bufs=2))

        # ---- load phase ------------------------------------------------
        # small constants first so projections can start the moment the
        # bulk x chunks land
        wq_sb = consts.tile([_P, CJ, _C8], f32r)
        wk_sb = consts.tile([_P, CJ, _C8], f32r)
        wv_sb = consts.tile([_P, CJ, _C], bf16)
        for j in range(CJ):
            nc.sync.dma_start(out=wq_sb[:, j, :], in_=wq[j * _P:(j + 1) * _P, :])
            nc.sync.dma_start(out=wk_sb[:, j, :], in_=wk[j * _P:(j + 1) * _P, :])
            nc.sync.dma_start(out=wv_sb[:, j, :], in_=wv[j * _P:(j + 1) * _P, :])
        bq_sb = consts.tile([_P, 1], f32)
        bk_sb = consts.tile([_P, 1], f32)
        bv_sb = consts.tile([_P, _C], f32)
        mask_sb = consts.tile([_P, NCH], bf16)
        invgs_sb = consts.tile([_P, _P], f32)
        nc.sync.dma_start(out=bq_sb, in_=bq[:, :])
        nc.sync.dma_start(out=bk_sb, in_=bk[:, :])
        nc.sync.dma_start(out=bv_sb, in_=bv[:, :])
        nc.sync.dma_start(out=mask_sb, in_=mask[:, :])
        nc.sync.dma_start(out=invgs_sb, in_=invgs[:, :])

        # x arrives in 8 column-blocks so projections, the bf16 cast and the
        # first mq pass can all pipeline behind the DMA stream
        xr_sb = xfp.tile([_P, CJ, _N], f32r)
        xb_sb = xfp.tile([_P, CJ, _N], bf16)
        NB = _N // 512
        for nb in range(NB):
            for j in range(CJ):
                nc.sync.dma_start(
                    out=xr_sb[:, j, ts(nb, 512)],
                    in_=xf_r[j * _P:(j + 1) * _P, ts(nb, 512)],
                )
            # derive the bf16 copy on-device (DVE converts on copy),
            # saving 4 MiB of HBM input traffic
            nc.vector.tensor_copy(
                out=xb_sb[:, :, ts(nb, 512)],
                in_=xr_sb[:, :, ts(nb, 512)].bitcast(f32),
            )

        # k is stored row-PACKED for the QK matmuls: even n-chunks on
        # partitions 0..63, odd n-chunks on partitions 64..127, so two K=64
        # QK matmuls run CONCURRENTLY in separate PE row-groups (~1.9x).
        # qT is duplicated onto both partition halves to feed them.
        k2_sb = kvp.tile([_P, NCH // 2, _P], f32r)
        ktmp = kvp.tile([_P, NCH // 2, _P], f32r)
        qt_sb = kvp.tile([_P, _M], f32r)
        vt_sb = kvp.tile([_P, NCH, _C], bf16)

        # ---- projections ----------------------------------------------
        with tc.tile_pool(name="pp_proj", bufs=2, space="PSUM") as pp_proj:
            # k[o, n] = Wk @ xf + bk  (o on partitions)
            for t8 in range(_N // 512):
                ps = pp_proj.tile([_C8, 512], f32, tag="pk")
                for j in range(CJ):
                    nc.tensor.matmul(
                        ps,
                        wk_sb[:, j, :],
                        xr_sb[:, j, ts(t8, 512)],
                        start=(j == 0), stop=(j == CJ - 1),
                    )
                # scatter the 4 n-chunks of this psum tile into the packed
                # layout: even chunks straight to the low half, odd chunks
                # via a staging tile + partition-shifting SBUF DMA (compute
                # engines are lane-aligned and cannot shift partitions)
                for i in range(4):
                    c = 4 * t8 + i
                    if c % 2 == 0:
                        nc.scalar.add(k2_sb[0:_C8, c // 2, :],
                                      ps[:, ts(i, _P)], bk_sb[0:_C8])
                    else:
                        nc.scalar.add(ktmp[0:_C8, c // 2, :],
                                      ps[:, ts(i, _P)], bk_sb[0:_C8])
                        nc.sync.dma_start(
                            out=k2_sb[_C8:2 * _C8, c // 2, :],
                            in_=ktmp[0:_C8, c // 2, :],
                        )
            # qT[o, m] = Wq @ xf[:, :2048] + bq, duplicated onto both halves
            for t4 in range(_M // 512):
                ps = pp_proj.tile([_C8, 512], f32, tag="pk")
                for j in range(CJ):
                    nc.tensor.matmul(
                        ps,
                        wq_sb[:, j, :],
                        xr_sb[:, j, ts(t4, 512)],
                        start=(j == 0), stop=(j == CJ - 1),
                    )
                nc.scalar.add(qt_sb[0:_C8, ts(t4, 512)], ps, bq_sb[0:_C8])
                nc.sync.dma_start(
                    out=qt_sb[_C8:2 * _C8, ts(t4, 512)],
                    in_=qt_sb[0:_C8, ts(t4, 512)],
                )

        # ---- attention main loop --------------------------------------
        with tc.tile_pool(name="pp_e", bufs=4, space="PSUM") as pp_e, \
             tc.tile_pool(name="pp_att", bufs=4, space="PSUM") as pp_att:
            for mq in range(MQ):
                atts = [
                    pp_att.tile([_P, 512], f32, tag="att", name=f"att_{mq}_{cc}")
                    for cc in range(CJ)
                ]
                # rowsum accumulators, split across DVE (even chunks) and the
                # otherwise-idle GpSimd (odd chunks) so neither chain paces
                # the loop
                acc_d = accp.tile([_P, 512], f32, tag="acc_d")
                acc_g = accp.tile([_P, 512], f32, tag="acc_g")
                nc.vector.memset(acc_d, 0.0)
                nc.gpsimd.memset(acc_g, 0.0)
                for tp in range(NCH // 2):
                    if mq == 0:
                        # vT[n, c] = xf^T @ Wv^T + bv, produced just-in-time
                        # for this pair's AV matmuls (hides the whole vT
                        # projection inside the first mq pass)
                        for t in (2 * tp, 2 * tp + 1):
                            vps = pp_e.tile([_P, _C], f32, tag="eps",
                                            name=f"vps_{t}")
                            for j in range(CJ):
                                nc.tensor.matmul(
                                    vps,
                                    xb_sb[:, j, ts(t, _P)],
                                    wv_sb[:, j, :],
                                    start=(j == 0), stop=(j == CJ - 1),
                                )
                            nc.vector.tensor_add(vt_sb[:, t, :], vps, bv_sb)
                    # two K=64 QK matmuls run concurrently in PE row-groups
                    # 0-1 (k2 low half) and 2-3 (k2 high half)
                    e_pair = []
                    for half in (0, 1):
                        e_ps = pp_e.tile([_P, 512], f32, tag="eps",
                                         name=f"eps_{mq}_{tp}_{half}")
                        nc.tensor.matmul(
                            e_ps,
                            k2_sb[half * _C8:(half + 1) * _C8, tp, :],
                            qt_sb[half * _C8:(half + 1) * _C8, ts(mq, 512)],
                            start=True, stop=True,
                        )
                        e_pair.append(e_ps)
                    for half in (0, 1):
                        t = 2 * tp + half
                        e_sb = work.tile([_P, 512], bf16, tag="esb", bufs=6)
                        nc.scalar.activation(e_sb, e_pair[half], Exp)
                        # rowsum accumulation must see E BEFORE masking
                        # (softmax denominator includes the masked element)
                        if half == 0:
                            nc.vector.tensor_add(acc_d, acc_d, e_sb)
                        else:
                            nc.gpsimd.tensor_add(acc_g, acc_g, e_sb)
                        if mq == 0:
                            # style mask: zero E[s_b, m=0] (data-driven;
                            # all-ones for cores not owning query row 0)
                            nc.vector.tensor_mul(
                                e_sb[:, 0:1], e_sb[:, 0:1], mask_sb[:, t:t + 1]
                            )
                        for cc in range(CJ):
                            nc.tensor.matmul(
                                atts[cc],
                                vt_sb[:, t, ts(cc, _P)],
                                e_sb,
                                start=(t == 0), stop=(t == NCH - 1),
                            )
                # Stage attended to SBUF on ScalarE immediately, freeing the
                # four PSUM banks so the next mq's AV matmuls aren't blocked
                # behind the (slow) normalization chain.
                att_cp = []
                for cc in range(CJ):
                    ac = outp.tile([_P, 512], f32, tag="att_cp",
                                   name=f"attcp_{mq}_{cc}", bufs=4)
                    nc.scalar.copy(ac, atts[cc])
                    att_cp.append(ac)
                # Fused partition-reduce + broadcast + un-scale:
                # rb[p, m] = sum_n invgs * (acc_d + acc_g)[n, m]
                #          = rowsum[m]/(gamma*NS)   (full fp32 matmuls)
                rb_ps = pp_e.tile([_P, 512], f32, tag="eps", name=f"rb_{mq}")
                nc.tensor.matmul(rb_ps, invgs_sb, acc_d, start=True, stop=False)
                nc.tensor.matmul(rb_ps, invgs_sb, acc_g, start=False, stop=True)
                recipb = smallp.tile([_P, 512], f32, tag="recipb")
                nc.vector.reciprocal(recipb, rb_ps)
                for cc in range(CJ):
                    o = outp.tile([_P, 512], f32, tag="o")
                    nc.vector.tensor_mul(o, att_cp[cc], recipb)
                    nc.vector.tensor_add(o, o, xr_sb[:, cc, ts(mq, 512)].bitcast(f32))
                    nc.sync.dma_start(
                        out=out[ts(cc, _P), ts(mq, 512)], in_=o
                    )

    _split_waits(nc, mybir)
    return nc


def _get_nc():
    if "nc" not in _NC_CACHE:
        _NC_CACHE["nc"] = _build_nc()
    return _NC_CACHE["nc"]


def _install_trace_shim():
    """Optional: enable NTFF profiling under axon (test.py sets
    KERNEL_TRACE=1). Registers the ctypes NTFF hook that the container's
    antenv lacks, and stubs out the artifact upload."""
    import sys, types
    if "antenv.axon_hooks" not in sys.modules:
        mod = types.ModuleType("antenv.axon_hooks")
        _h = [None]
        mod.set_axon_ntff_profile_hook = lambda h: _h.__setitem__(0, h)
        mod.get_axon_ntff_profile_hook = lambda: _h[0]
        sys.modules["antenv.axon_hooks"] = mod
    import antenv.axon_hooks as ah
    if ah.get_axon_ntff_profile_hook() is None:
        try:
            from trn_agent_boot.trn_boot import _ntff_profile_via_ctypes
            ah.set_axon_ntff_profile_hook(
                _ntff_profile_via_ctypes("/opt/axon/libaxon_pjrt.so")
            )
        except Exception:
            pass
    import concourse.bass_utils as bu
    bu.upload_artifacts = lambda tmpdir: "local://" + tmpdir


def kernel(x, Wq, bq, Wk, bk, Wv, bv, gamma, style_idx, num_styles):
    global LAST_EXEC_TIME_NS
    import ml_dtypes
    from concourse.bass_utils import run_bass_kernel_spmd

    x = np.asarray(x, dtype=np.float32)
    Wq = np.asarray(Wq, dtype=np.float32)
    Wk = np.asarray(Wk, dtype=np.float32)
    Wv = np.asarray(Wv, dtype=np.float32)
    bq = np.asarray(bq, dtype=np.float32).reshape(_C8, 1)
    bk = np.asarray(bk, dtype=np.float32).reshape(_C8, 1)
    bq = np.ascontiguousarray(np.vstack([bq, bq]))
    bk = np.ascontiguousarray(np.vstack([bk, bk]))
    bv = np.asarray(bv, dtype=np.float32).reshape(1, _C)
    gamma = np.asarray(gamma, dtype=np.float32).reshape(-1)
    style_idx = np.asarray(style_idx).reshape(-1).astype(np.int64)
    NS = int(np.asarray(num_styles))
    assert NS == _N, f"kernel specialized for num_styles == {_N}, got {NS}"
    B, C, H, W = x.shape
    assert (B, C, H, W) == (_B, _C, _H, _W)

    gscale = float(gamma[0]) * float(NS)
    inv = 1.0 / gscale if gscale != 0.0 else 1e30
    invgs = np.full((_P, _P), inv, dtype=np.float32)
    wqT = np.ascontiguousarray(Wq.T)                          # [C, C8] f32
    wkT = np.ascontiguousarray(Wk.T)
    wvT = np.ascontiguousarray(Wv.T.astype(ml_dtypes.bfloat16))
    bv128 = np.ascontiguousarray(np.broadcast_to(bv, (_P, _C)))

    xf_full = x.reshape(B, C, _N)
    in_maps = []
    for core in range(8):
        b, half = divmod(core, 2)
        xr = xf_full[b]
        if half:
            xr = np.concatenate([xr[:, _M:], xr[:, :_M]], axis=1)
        xr = np.ascontiguousarray(xr)
        m = np.ones((_P, _N // _P), dtype=np.float32)
        if half == 0:
            s = int(style_idx[b])
            m[s % _P, s // _P] = 0.0
        in_maps.append({
            "xf_r": xr,
            "wq_t": wqT, "wk_t": wkT, "wv_t": wvT,
            "bq": bq, "bk": bk, "bv128": bv128,
            "mask": m.astype(ml_dtypes.bfloat16), "invgs": invgs,
        })

    trace = os.environ.get("KERNEL_TRACE", "0") == "1"
    if trace:
        _install_trace_shim()

    nc = _get_nc()
    kw = {}
    tdir = os.environ.get("KERNEL_TRACE_DIR")
    if trace and tdir:
        os.makedirs(tdir, exist_ok=True)
        kw["tmpdir"] = tdir
    res = run_bass_kernel_spmd(
        nc, in_maps, core_ids=list(range(8)), trace=trace, **kw
    )
    LAST_EXEC_TIME_NS = res.exec_time_ns

    out = np.empty((B, C, _N), dtype=np.float32)
    for core in range(8):
        b, half = divmod(core, 2)
        out[b, :, half * _M:(half + 1) * _M] = res.results[core]["out_cm"]
    return out.reshape(B, C, H, W)
